# revision 19
# baseline (speedup 1.0000x reference)
"""Trainium2 Bass kernel for the ActionVQVAE forward pass.

Data-parallel across 8 NeuronCores: the batch (131072 rows) is split into 8
shards of 16384 rows; all weights are replicated. Each core runs encoder MLP ->
VQ argmin -> gather -> loss partials. The decoder is precomputed once per core
on the 512 codebook rows (its input is always a codebook row), so the per-row
decoder collapses into the same gather that produces q_st.

Matmuls use bf16 hi/lo splitting (3 bf16 products, fp32 PSUM accumulate) for
~2^-17 relative error at ~4x the speed of native fp32 matmuls. The VQ argmin
tolerates that error level: top-2 score gaps are quantized by the reference's
own fp32 rounding at ~1.5e-8, so a handful of tie-flips are unavoidable for
any implementation.

Outputs per core: idx shard (int32), q_st shard (f32), and per-partition loss
partials which the host sums into the scalar total_loss during unsharding.
"""

import numpy as np

B_TOTAL = 131072
A = 6            # action dim
H = 256          # hidden
D = 128          # latent
KCB = 512        # codebook size
NCORES = 8
BLOC = B_TOTAL // NCORES   # rows per core
RB = 512                   # rows per block
PCH = 128                  # rows per chunk (psum partition dim)
NCH = RB // PCH            # chunks per block
RPAD = 16                  # padded recons width (6 used)
TW = D + RPAD              # gather-table row width (f32 elements)
BETA = 0.25

_NC_CACHE = {}


def _split_bf16(x):
    import ml_dtypes
    bf = ml_dtypes.bfloat16
    hi = x.astype(bf)
    lo = (x.astype(np.float32) - hi.astype(np.float32)).astype(bf)
    return hi, lo


def _build_nc(bloc):
    from concourse import bacc, bass, mybir, tile

    f32 = mybir.dt.float32
    bf16 = mybir.dt.bfloat16
    u32 = mybir.dt.uint32
    i32 = mybir.dt.int32
    AX = mybir.AxisListType
    AF = mybir.ActivationFunctionType
    OP = mybir.AluOpType

    nblk = bloc // RB
    nc = bacc.Bacc(None, target_bir_lowering=False)

    def dp(name, shape, dt=f32, out=False):
        return nc.declare_dram_parameter(name, list(shape), dt, isOutput=out)

    act_d = dp("act", (bloc, A))
    actTh_d = dp("actTh", (A, bloc), bf16)
    actTl_d = dp("actTl", (A, bloc), bf16)
    w1Th_d = dp("w1Th", (A, H), bf16)
    w1Tl_d = dp("w1Tl", (A, H), bf16)
    w2Th_d = dp("w2Th", (H, H), bf16)
    w2Tl_d = dp("w2Tl", (H, H), bf16)
    muTh_d = dp("muTh", (H, D), bf16)
    muTl_d = dp("muTl", (H, D), bf16)
    embTh_d = dp("embTh", (D, KCB), bf16)
    embTl_d = dp("embTl", (D, KCB), bf16)
    b1_d = dp("b1", (H,))
    b2_d = dp("b2", (H,))
    bm_d = dp("bm", (D,))
    embTf_d = dp("embTf", (D, KCB))     # f32 embT for table/embsq math
    embN_d = dp("embN", (KCB, D))
    d1T_d = dp("d1T", (D, H))
    bd1_d = dp("bd1", (H,))
    d2T_d = dp("d2T", (H, H))
    bd2_d = dp("bd2", (H,))
    rT_d = dp("rT", (H, RPAD))
    br_d = dp("br", (RPAD,))
    idx_d = dp("idx_o", (bloc,), i32, True)
    q_d = dp("q_o", (bloc, D), f32, True)
    par_d = dp("partials", (PCH, 3), f32, True)

    with tile.TileContext(nc) as tc:
        with (
            tc.tile_pool(name="wp", bufs=1) as wp,
            tc.tile_pool(name="av", bufs=3) as av,
            tc.tile_pool(name="scr", bufs=1) as scr,
            tc.tile_pool(name="stg", bufs=1) as stg,
            tc.tile_pool(name="pm", bufs=1, space="PSUM") as pm,
            tc.tile_pool(name="dr", bufs=1, space="DRAM") as dr,
        ):
            table = dr.tile([KCB, TW], f32)

            # ---------- preamble: weights into SBUF ----------
            # enc1 K-stacking: lhsT [whi; whi] (12, H) with rhs [xhi; xlo]
            # computes whi^T(xhi+xlo); plus wlo^T xhi for the third product.
            w1s = wp.tile([2 * A, H], bf16)
            nc.scalar.dma_start(w1s[0:A, :], w1Th_d[:])
            nc.scalar.dma_start(w1s[A:2 * A, :], w1Th_d[:])
            w1l = wp.tile([A, H], bf16)
            nc.scalar.dma_start(w1l[:], w1Tl_d[:])

            def load_pair(ph, pl, shape, nm):
                nch = (shape[0] + 127) // 128
                out = []
                for k in range(nch):
                    rs = slice(k * 128, min((k + 1) * 128, shape[0]))
                    cshape = [rs.stop - rs.start, shape[1]]
                    th = wp.tile(cshape, bf16, name=f"{nm}h{k}")
                    tl = wp.tile(cshape, bf16, name=f"{nm}l{k}")
                    nc.scalar.dma_start(th[:], ph[rs, :])
                    nc.scalar.dma_start(tl[:], pl[rs, :])
                    out.append((th, tl))
                return out

            w2T = load_pair(w2Th_d, w2Tl_d, (H, H), "w2T")
            muT = load_pair(muTh_d, muTl_d, (H, D), "muT")
            embT = load_pair(embTh_d, embTl_d, (D, KCB), "embT")[0]
            embTf = wp.tile([D, KCB], f32)
            nc.scalar.dma_start(embTf[:], embTf_d[:])
            d1T = wp.tile([D, H], f32)
            nc.scalar.dma_start(d1T[:], d1T_d[:])
            d2T = []
            rT = []
            for j in range(2):
                t = wp.tile([PCH, H], f32, name=f"d2T{j}")
                nc.scalar.dma_start(t[:], d2T_d[j * 128:(j + 1) * 128, :])
                d2T.append(t)
                t = wp.tile([PCH, RPAD], f32, name=f"rT{j}")
                nc.scalar.dma_start(t[:], rT_d[j * 128:(j + 1) * 128, :])
                rT.append(t)

            def bias_tiles(dram, n, nm):
                out = []
                for j in range(n):
                    t = wp.tile([PCH, 1], f32, name=f"{nm}{j}")
                    nc.scalar.dma_start(t[:], dram[j * 128:(j + 1) * 128, None])
                    out.append(t)
                return out

            b1t = bias_tiles(b1_d, 2, "b1t")
            b2t = bias_tiles(b2_d, 2, "b2t")
            bd1t = bias_tiles(bd1_d, 2, "bd1t")
            bd2t = bias_tiles(bd2_d, 2, "bd2t")
            bmt = wp.tile([D, 1], f32)
            nc.scalar.dma_start(bmt[:], bm_d[:, None])
            brt = wp.tile([RPAD, 1], f32)
            nc.scalar.dma_start(brt[:], br_d[:, None])

            ones_col = wp.tile([D, 1], f32)
            nc.gpsimd.memset(ones_col[:], 1.0)
            ones2_bf = wp.tile([2, PCH], bf16)
            nc.gpsimd.memset(ones2_bf[:], 1.0)

            # ---------- score bias: -0.5*|emb_k|^2 as bf16 hi+lo rows ----------
            sqE = scr.tile([D, KCB], f32)
            nc.scalar.activation(sqE[:], embTf[:], AF.Square)
            esq_ps = pm.tile([1, KCB], f32, tag="s", bufs=3)
            nc.tensor.matmul(esq_ps[:], ones_col[:], sqE[:], start=True, stop=True)
            embsq_row = wp.tile([1, KCB], f32)
            nc.scalar.activation(embsq_row[:], esq_ps[:], AF.Copy, scale=-0.5)
            bias_hi_row = scr.tile([1, KCB], bf16)
            nc.scalar.activation(bias_hi_row[:], embsq_row[:], AF.Copy)
            bias_lo_f = scr.tile([1, KCB], f32)
            nc.vector.tensor_tensor(out=bias_lo_f[:], in0=embsq_row[:],
                                    in1=bias_hi_row[:], op=OP.subtract)
            bias_lo_row = scr.tile([1, KCB], bf16)
            nc.vector.tensor_copy(out=bias_lo_row[:], in_=bias_lo_f[:])
            bias_hl = wp.tile([2, KCB], bf16)
            nc.scalar.dma_start(bias_hl[0:1, :], bias_hi_row[:])
            nc.scalar.dma_start(bias_hl[1:2, :], bias_lo_row[:])

            # ---------- decoder precompute on the 512 codebook rows ----------
            t1 = []
            for j in range(2):
                ps = pm.tile([PCH, KCB], f32, tag="s", bufs=3)
                nc.tensor.matmul(ps[:], d1T[:, j * 128:(j + 1) * 128], embTf[:],
                                 start=True, stop=True)
                t = wp.tile([PCH, KCB], f32, name=f"t1_{j}")
                nc.scalar.activation(t[:], ps[:], AF.Relu, bias=bd1t[j][:])
                t1.append(t)
            t2 = []
            for j in range(2):
                ps = pm.tile([PCH, KCB], f32, tag="s", bufs=3)
                for k in range(2):
                    nc.tensor.matmul(ps[:], d2T[k][:, j * 128:(j + 1) * 128], t1[k][:],
                                     start=(k == 0), stop=(k == 1))
                t = wp.tile([PCH, KCB], f32, name=f"t2_{j}")
                nc.scalar.activation(t[:], ps[:], AF.Relu, bias=bd2t[j][:])
                t2.append(t)
            rec_ps = pm.tile([RPAD, KCB], f32, tag="s", bufs=3)
            for k in range(2):
                nc.tensor.matmul(rec_ps[:], rT[k][:], t2[k][:],
                                 start=(k == 0), stop=(k == 1))
            recT = wp.tile([RPAD, KCB], f32)
            nc.scalar.activation(recT[:], rec_ps[:], AF.Tanh, bias=brt[:])

            # ---------- assemble gather table in DRAM ----------
            embB = scr.tile([PCH, KCB // PCH, D], f32)
            nc.scalar.dma_start(embB[:], embN_d[:].rearrange("(c p) d -> p c d", p=PCH))
            nc.scalar.dma_start(table[:, 0:D].rearrange("(c p) d -> p c d", p=PCH), embB[:])
            nc.scalar.dma_start(table[:, D:TW].rearrange("k a -> a k"), recT[:])

            # ---------- staging ----------
            esq_st = stg.tile([PCH, nblk], f32)
            rsq_st = stg.tile([PCH, nblk], f32)
            mvr_st = stg.tile([PCH, nblk], f32)

            def emit_group(ps, pairs):
                n = len(pairs)
                for i, (lw, rx) in enumerate(pairs):
                    nc.tensor.matmul(ps[:], lw, rx, start=(i == 0),
                                     stop=(i == n - 1))

            def relu_split(ps, bias_ap, nm):
                """PSUM -> (hi bf16, lo bf16). hi first so the two hi-products
                of the next layer can start before lo lands."""
                th = av.tile([PCH, RB], bf16, tag=nm + "h")
                nc.scalar.activation(th[:], ps[:], AF.Relu, bias=bias_ap)
                tf = av.tile([PCH, RB], f32, tag=nm + "f")
                nc.scalar.activation(tf[:], ps[:], AF.Relu, bias=bias_ap)
                tl = av.tile([PCH, RB], bf16, tag=nm + "l")
                nc.vector.tensor_tensor(out=tl[:], in0=tf[:], in1=th[:],
                                        op=OP.subtract)
                return th, tl

            def emit_dma_in(b):
                r0 = b * RB
                xhl = av.tile([2 * A, RB], bf16, tag="xhl", bufs=4)
                nc.sync.dma_start(xhl[0:A, :], actTh_d[:, r0:r0 + RB])
                nc.sync.dma_start(xhl[A:2 * A, :], actTl_d[:, r0:r0 + RB])
                actn = av.tile([PCH, NCH, A], f32, tag="actn", bufs=5)
                nc.sync.dma_start(
                    actn[:], act_d[r0:r0 + RB, :].rearrange("(c p) a -> p c a", p=PCH))
                return xhl, actn

            def emit_enc1(b, xhl):
                h1 = []
                for j in range(2):
                    ps = pm.tile([PCH, RB], f32, tag=f"h1_{j}")
                    jc = slice(j * 128, (j + 1) * 128)
                    emit_group(ps, [(w1s[:, jc], xhl[:]),
                                    (w1l[:, jc], xhl[0:A, :])])
                    h1.append(relu_split(ps, b1t[j][:], f"h1{j}"))
                return h1

            def emit_enc2(b, h1):
                h2 = []
                for j in range(2):
                    ps = pm.tile([PCH, RB], f32, tag=f"h2_{j}")
                    jc = slice(j * 128, (j + 1) * 128)
                    pairs = []
                    for k in range(2):
                        wh, wl = w2T[k]
                        xh, xl = h1[k]
                        pairs += [(wh[:, jc], xh[:]), (wl[:, jc], xh[:]),
                                  (wh[:, jc], xl[:])]
                    # hi-only products first so they can start before los land
                    pairs = [pairs[0], pairs[1], pairs[3], pairs[4],
                             pairs[2], pairs[5]]
                    emit_group(ps, pairs)
                    h2.append(relu_split(ps, b2t[j][:], f"h2{j}"))
                return h2

            def emit_mu(b, h2):
                encps = pm.tile([PCH, RB], f32, tag="enc")
                pairs = []
                for k in range(2):
                    wh, wl = muT[k]
                    xh, xl = h2[k]
                    pairs += [(wh[:], xh[:]), (wl[:], xh[:]), (wh[:], xl[:])]
                pairs = [pairs[0], pairs[1], pairs[3], pairs[4],
                         pairs[2], pairs[5]]
                emit_group(encps, pairs)
                # enc splits straight from PSUM (no relu on the mu layer)
                ench = av.tile([PCH, RB], bf16, tag="ench", bufs=4)
                nc.scalar.activation(ench[:], encps[:], AF.Identity, bias=bmt[:])
                sq_scr = scr.tile([PCH, RB], f32, tag="sqscr")
                nc.scalar.activation(sq_scr[:], encps[:], AF.Square, bias=bmt[:],
                                     accum_out=esq_st[:, b:b + 1])
                encl = av.tile([PCH, RB], bf16, tag="encl", bufs=4)
                nc.vector.scalar_tensor_tensor(out=encl[:], in0=encps[:],
                                               scalar=bmt[:], in1=ench[:],
                                               op0=OP.add, op1=OP.subtract)
                return ench, encl

            def emit_vq_chunks(st, crange):
                b, ench, encl, actn, mv, ib = st
                for c in crange:
                    cs = slice(c * 128, (c + 1) * 128)
                    sps = pm.tile([PCH, KCB], f32, tag="s", bufs=3)
                    emit_group(sps, [(ench[:, cs], embT[0][:]),
                                     (ench[:, cs], embT[1][:]),
                                     (encl[:, cs], embT[0][:]),
                                     (ones2_bf[:], bias_hl[:])])
                    mv8 = mv[:].rearrange("p (e c) -> p c e", c=NCH)[:, c, :]
                    ib8 = ib[:].rearrange("p (e c) -> p c e", c=NCH)[:, c, :]
                    nc.vector.max(mv8, sps[:])
                    nc.vector.max_index(ib8, mv8, sps[:])

            def emit_gathers(st):
                b, ench, encl, actn, mv, ib = st
                gt = av.tile([PCH, NCH, TW], f32, tag="gath", bufs=4)
                for c in range(NCH):
                    # HW indirect DMA supports one offset per partition
                    nc.gpsimd.indirect_dma_start(
                        out=gt[:, c, :], out_offset=None,
                        in_=table[:],
                        in_offset=bass.IndirectOffsetOnAxis(ap=ib[:, c:c + 1], axis=0),
                    )
                return st + (gt,)

            def emit_drains(st):
                """Output-side work for a block whose gathers finished long
                ago — emitted late so no engine queue ever blocks on it."""
                b, ench, encl, actn, mv, ib, gt = st
                r0 = b * RB
                nc.vector.tensor_reduce(out=mvr_st[:, b:b + 1], in_=mv[:, 0:NCH],
                                        axis=AX.X, op=OP.add)
                nc.sync.dma_start(
                    q_d[r0:r0 + RB, :].rearrange("(c p) d -> p c d", p=PCH),
                    gt[:, :, 0:D])
                nc.sync.dma_start(
                    idx_d[r0:r0 + RB].rearrange("(c p) -> p c", p=PCH),
                    ib[:, 0:NCH].bitcast(i32))
                t6 = scr.tile([PCH, NCH, A], f32, tag="t6", bufs=2)
                nc.vector.tensor_tensor(out=t6[:], in0=gt[:, :, D:D + A], in1=actn[:],
                                        op=OP.subtract)
                sq6 = scr.tile([PCH, NCH * A], f32, tag="sq6", bufs=2)
                nc.scalar.activation(sq6[:], t6[:].rearrange("p c a -> p (c a)"),
                                     AF.Square, accum_out=rsq_st[:, b:b + 1])

            # ---------- main loop, layer-interleaved software pipeline ----------
            # While block b's encoder relays through ACT/DVE between layers,
            # the PE works on the VQ chunk matmuls of a block PDEPTH behind,
            # whose inputs are long since ready. Output-side drains trail one
            # further stage behind.
            PDEPTH = 2
            pend_vq = []
            pend_drain = []
            for b in range(nblk):
                xhl, actn = emit_dma_in(b)
                h1 = emit_enc1(b, xhl)
                if len(pend_vq) >= PDEPTH:
                    emit_vq_chunks(pend_vq[0], range(0, 2))
                h2 = emit_enc2(b, h1)
                if len(pend_vq) >= PDEPTH:
                    st = pend_vq.pop(0)
                    emit_vq_chunks(st, range(2, NCH))
                    pend_drain.append(emit_gathers(st))
                ench, encl = emit_mu(b, h2)
                if len(pend_drain) > 1:
                    emit_drains(pend_drain.pop(0))
                mv = av.tile([PCH, NCH * 8], f32, tag="mv",
                             bufs=PDEPTH + 2)   # col layout e*NCH+c
                ib = av.tile([PCH, NCH * 8], u32, tag="ib", bufs=PDEPTH + 2)
                pend_vq.append((b, ench, encl, actn, mv, ib))
            for st in pend_vq:
                emit_vq_chunks(st, range(0, NCH))
                pend_drain.append(emit_gathers(st))
            for st in pend_drain:
                emit_drains(st)

            # ---------- partials ----------
            pc = stg.tile([PCH, 3], f32)
            nc.vector.tensor_reduce(out=pc[:, 0:1], in_=esq_st[:], axis=AX.X, op=OP.add)
            nc.vector.tensor_reduce(out=pc[:, 1:2], in_=mvr_st[:], axis=AX.X, op=OP.add)
            nc.vector.tensor_reduce(out=pc[:, 2:3], in_=rsq_st[:], axis=AX.X, op=OP.add)
            nc.sync.dma_start(par_d[:], pc[:])

    nc.compile()
    return nc


def _make_in_map(core, action, enc1_w, enc1_b, enc2_w, enc2_b, mu_w, mu_b, emb,
                 dec1_w, dec1_b, dec2_w, dec2_b, rec_w, rec_b, bloc=BLOC):
    c = np.ascontiguousarray
    sl = slice(core * bloc, (core + 1) * bloc)
    act = c(action[sl]).astype(np.float32)
    rT = np.zeros((H, RPAD), np.float32)
    rT[:, :A] = rec_w.T
    br = np.zeros((RPAD,), np.float32)
    br[:A] = rec_b
    m = {
        "act": act,
        "b1": c(enc1_b),
        "b2": c(enc2_b),
        "bm": c(mu_b),
        "embTf": c(emb.T),
        "embN": c(emb),
        "d1T": c(dec1_w.T),
        "bd1": c(dec1_b),
        "d2T": c(dec2_w.T),
        "bd2": c(dec2_b),
        "rT": rT,
        "br": br,
    }
    hi, lo = _split_bf16(c(act.T))
    m["actTh"], m["actTl"] = c(hi), c(lo)
    for nm, w in [("w1T", enc1_w.T), ("w2T", enc2_w.T), ("muT", mu_w.T),
                  ("embT", emb.T)]:
        hi, lo = _split_bf16(c(w.astype(np.float32)))
        m[nm + "h"], m[nm + "l"] = c(hi), c(lo)
    return m


def kernel(action, enc1_w, enc1_b, enc2_w, enc2_b, mu_w, mu_b, emb,
           dec1_w, dec1_b, dec2_w, dec2_b, rec_w, rec_b,
           _trace=False, _result_hook=None):
    from concourse.bass_utils import run_bass_kernel_spmd

    if BLOC not in _NC_CACHE:
        _NC_CACHE[BLOC] = _build_nc(BLOC)
    nc = _NC_CACHE[BLOC]

    args = (action, enc1_w, enc1_b, enc2_w, enc2_b, mu_w, mu_b, emb,
            dec1_w, dec1_b, dec2_w, dec2_b, rec_w, rec_b)
    in_maps = [_make_in_map(core, *args) for core in range(NCORES)]
    res = run_bass_kernel_spmd(nc, in_maps, core_ids=list(range(NCORES)),
                               trace=_trace)
    if _result_hook is not None:
        _result_hook(res)

    idx = np.concatenate([res.results[c]["idx_o"] for c in range(NCORES)])
    q_st = np.concatenate([res.results[c]["q_o"] for c in range(NCORES)])
    par = np.stack([res.results[c]["partials"] for c in range(NCORES)])
    # partials: [:, :, 0]=sum(enc^2), [:, :, 1]=sum(maxval), [:, :, 2]=sum((rec-act)^2)
    esum = float(par[:, :, 0].astype(np.float64).sum())
    msum = float(par[:, :, 1].astype(np.float64).sum())
    rsum = float(par[:, :, 2].astype(np.float64).sum())
    vq_sum = esum - 2.0 * msum          # sum over rows of min squared distance
    recons_loss = rsum / (B_TOTAL * A)
    vq_loss = (1.0 + BETA) * vq_sum / (B_TOTAL * D)
    total_loss = np.float32(recons_loss + vq_loss)
    return idx.astype(np.int32), q_st.astype(np.float32), total_loss


# revision 20
# speedup vs baseline: 1.1407x; 1.1407x over previous
"""Trainium2 Bass kernel for the ActionVQVAE forward pass.

Data-parallel across 8 NeuronCores: the batch (131072 rows) is split into 8
shards of 16384 rows; all weights are replicated. Each core runs encoder MLP ->
VQ argmin -> gather -> loss partials. The decoder is precomputed once per core
on the 512 codebook rows (its input is always a codebook row), so the per-row
decoder collapses into the same gather that produces q_st.

Matmuls use bf16 hi/lo splitting (3 bf16 products, fp32 PSUM accumulate) for
~2^-17 relative error at ~4x the speed of native fp32 matmuls. The VQ argmin
tolerates that error level: top-2 score gaps are quantized by the reference's
own fp32 rounding at ~1.5e-8, so a handful of tie-flips are unavoidable for
any implementation.

Outputs per core: idx shard (int32), q_st shard (f32), and per-partition loss
partials which the host sums into the scalar total_loss during unsharding.
"""

import numpy as np

B_TOTAL = 131072
A = 6            # action dim
H = 256          # hidden
D = 128          # latent
KCB = 512        # codebook size
NCORES = 8
BLOC = B_TOTAL // NCORES   # rows per core
RB = 512                   # rows per block
PCH = 128                  # rows per chunk (psum partition dim)
NCH = RB // PCH            # chunks per block
RPAD = 16                  # padded recons width (6 used)
TW = D + RPAD              # gather-table row width (f32 elements)
BETA = 0.25

_NC_CACHE = {}


def _split_bf16(x):
    import ml_dtypes
    bf = ml_dtypes.bfloat16
    hi = x.astype(bf)
    lo = (x.astype(np.float32) - hi.astype(np.float32)).astype(bf)
    return hi, lo


def _build_nc(bloc):
    from concourse import bacc, bass, mybir, tile

    f32 = mybir.dt.float32
    bf16 = mybir.dt.bfloat16
    u32 = mybir.dt.uint32
    i32 = mybir.dt.int32
    AX = mybir.AxisListType
    AF = mybir.ActivationFunctionType
    OP = mybir.AluOpType

    nblk = bloc // RB
    nc = bacc.Bacc(None, target_bir_lowering=False)

    def dp(name, shape, dt=f32, out=False):
        return nc.declare_dram_parameter(name, list(shape), dt, isOutput=out)

    act_d = dp("act", (bloc, A))
    actTh_d = dp("actTh", (A, bloc), bf16)
    actTl_d = dp("actTl", (A, bloc), bf16)
    w1Th_d = dp("w1Th", (A, H), bf16)
    w1Tl_d = dp("w1Tl", (A, H), bf16)
    w2Th_d = dp("w2Th", (H, H), bf16)
    w2Tl_d = dp("w2Tl", (H, H), bf16)
    muTh_d = dp("muTh", (H, D), bf16)
    muTl_d = dp("muTl", (H, D), bf16)
    embTh_d = dp("embTh", (D, KCB), bf16)
    embTl_d = dp("embTl", (D, KCB), bf16)
    b1_d = dp("b1", (H,))
    b2_d = dp("b2", (H,))
    bm_d = dp("bm", (D,))
    embTf_d = dp("embTf", (D, KCB))     # f32 embT for table/embsq math
    embN_d = dp("embN", (KCB, D))
    d1T_d = dp("d1T", (D, H))
    bd1_d = dp("bd1", (H,))
    d2T_d = dp("d2T", (H, H))
    bd2_d = dp("bd2", (H,))
    rT_d = dp("rT", (H, RPAD))
    br_d = dp("br", (RPAD,))
    idx_d = dp("idx_o", (bloc,), i32, True)
    q_d = dp("q_o", (bloc, D), f32, True)
    par_d = dp("partials", (PCH, 3), f32, True)

    with tile.TileContext(nc) as tc:
        with (
            tc.tile_pool(name="wp", bufs=1) as wp,
            tc.tile_pool(name="av", bufs=2) as av,
            tc.tile_pool(name="scr", bufs=1) as scr,
            tc.tile_pool(name="stg", bufs=1) as stg,
            tc.tile_pool(name="pm", bufs=1, space="PSUM") as pm,
            tc.tile_pool(name="dr", bufs=1, space="DRAM") as dr,
        ):
            table = dr.tile([KCB, TW], f32)

            # ---------- preamble: weights into SBUF ----------
            # enc1 K-stacking: lhsT [whi; whi] (12, H) with rhs [xhi; xlo]
            # computes whi^T(xhi+xlo); plus wlo^T xhi for the third product.
            w1s = wp.tile([2 * A, H], bf16)
            nc.scalar.dma_start(w1s[0:A, :], w1Th_d[:])
            nc.scalar.dma_start(w1s[A:2 * A, :], w1Th_d[:])
            w1l = wp.tile([A, H], bf16)
            nc.scalar.dma_start(w1l[:], w1Tl_d[:])

            def load_pair(ph, pl, shape, nm):
                nch = (shape[0] + 127) // 128
                out = []
                for k in range(nch):
                    rs = slice(k * 128, min((k + 1) * 128, shape[0]))
                    cshape = [rs.stop - rs.start, shape[1]]
                    th = wp.tile(cshape, bf16, name=f"{nm}h{k}")
                    tl = wp.tile(cshape, bf16, name=f"{nm}l{k}")
                    nc.scalar.dma_start(th[:], ph[rs, :])
                    nc.scalar.dma_start(tl[:], pl[rs, :])
                    out.append((th, tl))
                return out

            w2T = load_pair(w2Th_d, w2Tl_d, (H, H), "w2T")
            muT = load_pair(muTh_d, muTl_d, (H, D), "muT")
            embT = load_pair(embTh_d, embTl_d, (D, KCB), "embT")[0]
            embTf = wp.tile([D, KCB], f32)
            nc.scalar.dma_start(embTf[:], embTf_d[:])
            d1T = wp.tile([D, H], f32)
            nc.scalar.dma_start(d1T[:], d1T_d[:])
            d2T = []
            rT = []
            for j in range(2):
                t = wp.tile([PCH, H], f32, name=f"d2T{j}")
                nc.scalar.dma_start(t[:], d2T_d[j * 128:(j + 1) * 128, :])
                d2T.append(t)
                t = wp.tile([PCH, RPAD], f32, name=f"rT{j}")
                nc.scalar.dma_start(t[:], rT_d[j * 128:(j + 1) * 128, :])
                rT.append(t)

            def bias_tiles(dram, n, nm):
                out = []
                for j in range(n):
                    t = wp.tile([PCH, 1], f32, name=f"{nm}{j}")
                    nc.scalar.dma_start(t[:], dram[j * 128:(j + 1) * 128, None])
                    out.append(t)
                return out

            b1t = bias_tiles(b1_d, 2, "b1t")
            b2t = bias_tiles(b2_d, 2, "b2t")
            bd1t = bias_tiles(bd1_d, 2, "bd1t")
            bd2t = bias_tiles(bd2_d, 2, "bd2t")
            bmt = wp.tile([D, 1], f32)
            nc.scalar.dma_start(bmt[:], bm_d[:, None])
            brt = wp.tile([RPAD, 1], f32)
            nc.scalar.dma_start(brt[:], br_d[:, None])

            ones_col = wp.tile([D, 1], f32)
            nc.gpsimd.memset(ones_col[:], 1.0)
            ones2_bf = wp.tile([2, PCH], bf16)
            nc.gpsimd.memset(ones2_bf[:], 1.0)

            # ---------- score bias: -0.5*|emb_k|^2 as bf16 hi+lo rows ----------
            sqE = scr.tile([D, KCB], f32)
            nc.scalar.activation(sqE[:], embTf[:], AF.Square)
            esq_ps = pm.tile([1, KCB], f32, tag="s", bufs=3)
            nc.tensor.matmul(esq_ps[:], ones_col[:], sqE[:], start=True, stop=True)
            embsq_row = wp.tile([1, KCB], f32)
            nc.scalar.activation(embsq_row[:], esq_ps[:], AF.Copy, scale=-0.5)
            bias_hi_row = scr.tile([1, KCB], bf16)
            nc.scalar.activation(bias_hi_row[:], embsq_row[:], AF.Copy)
            bias_lo_f = scr.tile([1, KCB], f32)
            nc.vector.tensor_tensor(out=bias_lo_f[:], in0=embsq_row[:],
                                    in1=bias_hi_row[:], op=OP.subtract)
            bias_lo_row = scr.tile([1, KCB], bf16)
            nc.vector.tensor_copy(out=bias_lo_row[:], in_=bias_lo_f[:])
            bias_hl = wp.tile([2, KCB], bf16)
            nc.scalar.dma_start(bias_hl[0:1, :], bias_hi_row[:])
            nc.scalar.dma_start(bias_hl[1:2, :], bias_lo_row[:])

            # ---------- decoder precompute on the 512 codebook rows ----------
            t1 = []
            for j in range(2):
                ps = pm.tile([PCH, KCB], f32, tag="s", bufs=3)
                nc.tensor.matmul(ps[:], d1T[:, j * 128:(j + 1) * 128], embTf[:],
                                 start=True, stop=True)
                t = wp.tile([PCH, KCB], f32, name=f"t1_{j}")
                nc.scalar.activation(t[:], ps[:], AF.Relu, bias=bd1t[j][:])
                t1.append(t)
            t2 = []
            for j in range(2):
                ps = pm.tile([PCH, KCB], f32, tag="s", bufs=3)
                for k in range(2):
                    nc.tensor.matmul(ps[:], d2T[k][:, j * 128:(j + 1) * 128], t1[k][:],
                                     start=(k == 0), stop=(k == 1))
                t = wp.tile([PCH, KCB], f32, name=f"t2_{j}")
                nc.scalar.activation(t[:], ps[:], AF.Relu, bias=bd2t[j][:])
                t2.append(t)
            rec_ps = pm.tile([RPAD, KCB], f32, tag="s", bufs=3)
            for k in range(2):
                nc.tensor.matmul(rec_ps[:], rT[k][:], t2[k][:],
                                 start=(k == 0), stop=(k == 1))
            recT = wp.tile([RPAD, KCB], f32)
            nc.scalar.activation(recT[:], rec_ps[:], AF.Tanh, bias=brt[:])

            # ---------- assemble gather table in DRAM ----------
            embB = scr.tile([PCH, KCB // PCH, D], f32)
            nc.scalar.dma_start(embB[:], embN_d[:].rearrange("(c p) d -> p c d", p=PCH))
            nc.scalar.dma_start(table[:, 0:D].rearrange("(c p) d -> p c d", p=PCH), embB[:])
            nc.scalar.dma_start(table[:, D:TW].rearrange("k a -> a k"), recT[:])

            # ---------- staging ----------
            esq_st = stg.tile([PCH, nblk], f32)
            rsq_st = stg.tile([PCH, nblk], f32)
            mvr_st = stg.tile([PCH, nblk], f32)

            def emit_group(ps, pairs):
                n = len(pairs)
                for i, (lw, rx) in enumerate(pairs):
                    nc.tensor.matmul(ps[:], lw, rx, start=(i == 0),
                                     stop=(i == n - 1))

            def relu_split(ps, bias_ap, nm):
                """PSUM -> (hi bf16, lo bf16). hi first so the two hi-products
                of the next layer can start before lo lands."""
                th = av.tile([PCH, RB], bf16, tag=nm + "h")
                nc.scalar.activation(th[:], ps[:], AF.Relu, bias=bias_ap)
                tf = av.tile([PCH, RB], f32, tag=nm + "f")
                nc.scalar.activation(tf[:], ps[:], AF.Relu, bias=bias_ap)
                tl = av.tile([PCH, RB], bf16, tag=nm + "l")
                nc.vector.tensor_tensor(out=tl[:], in0=tf[:], in1=th[:],
                                        op=OP.subtract)
                return th, tl

            def emit_dma_in(b):
                r0 = b * RB
                xhl = av.tile([2 * A, RB], bf16, tag="xhl", bufs=4)
                nc.sync.dma_start(xhl[0:A, :], actTh_d[:, r0:r0 + RB])
                nc.sync.dma_start(xhl[A:2 * A, :], actTl_d[:, r0:r0 + RB])
                actn = av.tile([PCH, NCH, A], f32, tag="actn", bufs=5)
                nc.sync.dma_start(
                    actn[:], act_d[r0:r0 + RB, :].rearrange("(c p) a -> p c a", p=PCH))
                return xhl, actn

            def emit_enc1(b, xhl):
                h1 = []
                for j in range(2):
                    ps = pm.tile([PCH, RB], f32, tag=f"h1_{j}")
                    jc = slice(j * 128, (j + 1) * 128)
                    emit_group(ps, [(w1s[:, jc], xhl[:]),
                                    (w1l[:, jc], xhl[0:A, :])])
                    h1.append(relu_split(ps, b1t[j][:], f"h1{j}"))
                return h1

            def emit_enc2(b, h1):
                h2 = []
                for j in range(2):
                    ps = pm.tile([PCH, RB], f32, tag=f"h2_{j}")
                    jc = slice(j * 128, (j + 1) * 128)
                    pairs = []
                    for k in range(2):
                        wh, wl = w2T[k]
                        xh, xl = h1[k]
                        pairs += [(wh[:, jc], xh[:]), (wl[:, jc], xh[:]),
                                  (wh[:, jc], xl[:])]
                    # hi-only products first so they can start before los land
                    pairs = [pairs[0], pairs[1], pairs[3], pairs[4],
                             pairs[2], pairs[5]]
                    emit_group(ps, pairs)
                    h2.append(relu_split(ps, b2t[j][:], f"h2{j}"))
                return h2

            def emit_mu(b, h2):
                encps = pm.tile([PCH, RB], f32, tag="enc")
                pairs = []
                for k in range(2):
                    wh, wl = muT[k]
                    xh, xl = h2[k]
                    pairs += [(wh[:], xh[:]), (wl[:], xh[:]), (wh[:], xl[:])]
                pairs = [pairs[0], pairs[1], pairs[3], pairs[4],
                         pairs[2], pairs[5]]
                emit_group(encps, pairs)
                # enc splits straight from PSUM (no relu on the mu layer)
                ench = av.tile([PCH, RB], bf16, tag="ench", bufs=4)
                nc.scalar.activation(ench[:], encps[:], AF.Identity, bias=bmt[:])
                sq_scr = scr.tile([PCH, RB], f32, tag="sqscr")
                nc.scalar.activation(sq_scr[:], encps[:], AF.Square, bias=bmt[:],
                                     accum_out=esq_st[:, b:b + 1])
                encl = av.tile([PCH, RB], bf16, tag="encl", bufs=4)
                nc.vector.scalar_tensor_tensor(out=encl[:], in0=encps[:],
                                               scalar=bmt[:], in1=ench[:],
                                               op0=OP.add, op1=OP.subtract)
                return ench, encl

            def emit_vq_chunks(st, crange):
                b, ench, encl, actn, mv, ib = st
                for c in crange:
                    cs = slice(c * 128, (c + 1) * 128)
                    sps = pm.tile([PCH, KCB], f32, tag="s", bufs=3)
                    emit_group(sps, [(ench[:, cs], embT[0][:]),
                                     (ench[:, cs], embT[1][:]),
                                     (encl[:, cs], embT[0][:]),
                                     (ones2_bf[:], bias_hl[:])])
                    mv8 = mv[:].rearrange("p (e c) -> p c e", c=NCH)[:, c, :]
                    ib8 = ib[:].rearrange("p (e c) -> p c e", c=NCH)[:, c, :]
                    nc.vector.max(mv8, sps[:])
                    nc.vector.max_index(ib8, mv8, sps[:])

            def emit_gathers(st):
                b, ench, encl, actn, mv, ib = st
                gt = av.tile([PCH, NCH, TW], f32, tag="gath", bufs=4)
                for c in range(NCH):
                    # HW indirect DMA supports one offset per partition
                    nc.gpsimd.indirect_dma_start(
                        out=gt[:, c, :], out_offset=None,
                        in_=table[:],
                        in_offset=bass.IndirectOffsetOnAxis(ap=ib[:, c:c + 1], axis=0),
                    )
                return st + (gt,)

            def emit_drains(st):
                """Output-side work for a block whose gathers finished long
                ago — emitted late so no engine queue ever blocks on it."""
                b, ench, encl, actn, mv, ib, gt = st
                r0 = b * RB
                nc.vector.tensor_reduce(out=mvr_st[:, b:b + 1], in_=mv[:, 0:NCH],
                                        axis=AX.X, op=OP.add)
                nc.sync.dma_start(
                    q_d[r0:r0 + RB, :].rearrange("(c p) d -> p c d", p=PCH),
                    gt[:, :, 0:D])
                nc.sync.dma_start(
                    idx_d[r0:r0 + RB].rearrange("(c p) -> p c", p=PCH),
                    ib[:, 0:NCH].bitcast(i32))
                t6 = scr.tile([PCH, NCH, A], f32, tag="t6", bufs=2)
                nc.vector.tensor_tensor(out=t6[:], in0=gt[:, :, D:D + A], in1=actn[:],
                                        op=OP.subtract)
                sq6 = scr.tile([PCH, NCH * A], f32, tag="sq6", bufs=2)
                nc.scalar.activation(sq6[:], t6[:].rearrange("p c a -> p (c a)"),
                                     AF.Square, accum_out=rsq_st[:, b:b + 1])

            # ---------- main loop, layer-interleaved software pipeline ----------
            # While block b's encoder relays through ACT/DVE between layers,
            # the PE works on the VQ chunk matmuls of a block PDEPTH behind,
            # whose inputs are long since ready. Output-side drains trail one
            # further stage behind.
            PDEPTH = 2
            pend_vq = []
            pend_drain = []
            for b in range(nblk):
                xhl, actn = emit_dma_in(b)
                h1 = emit_enc1(b, xhl)
                if len(pend_vq) >= PDEPTH:
                    emit_vq_chunks(pend_vq[0], range(0, 2))
                h2 = emit_enc2(b, h1)
                if len(pend_vq) >= PDEPTH:
                    st = pend_vq.pop(0)
                    emit_vq_chunks(st, range(2, NCH))
                    pend_drain.append(emit_gathers(st))
                ench, encl = emit_mu(b, h2)
                if len(pend_drain) > 1:
                    emit_drains(pend_drain.pop(0))
                mv = av.tile([PCH, NCH * 8], f32, tag="mv",
                             bufs=PDEPTH + 2)   # col layout e*NCH+c
                ib = av.tile([PCH, NCH * 8], u32, tag="ib", bufs=PDEPTH + 2)
                pend_vq.append((b, ench, encl, actn, mv, ib))
            for st in pend_vq:
                emit_vq_chunks(st, range(0, NCH))
                pend_drain.append(emit_gathers(st))
            for st in pend_drain:
                emit_drains(st)

            # ---------- partials ----------
            pc = stg.tile([PCH, 3], f32)
            nc.vector.tensor_reduce(out=pc[:, 0:1], in_=esq_st[:], axis=AX.X, op=OP.add)
            nc.vector.tensor_reduce(out=pc[:, 1:2], in_=mvr_st[:], axis=AX.X, op=OP.add)
            nc.vector.tensor_reduce(out=pc[:, 2:3], in_=rsq_st[:], axis=AX.X, op=OP.add)
            nc.sync.dma_start(par_d[:], pc[:])

    nc.compile()
    return nc


def _make_in_map(core, action, enc1_w, enc1_b, enc2_w, enc2_b, mu_w, mu_b, emb,
                 dec1_w, dec1_b, dec2_w, dec2_b, rec_w, rec_b, bloc=BLOC):
    c = np.ascontiguousarray
    sl = slice(core * bloc, (core + 1) * bloc)
    act = c(action[sl]).astype(np.float32)
    rT = np.zeros((H, RPAD), np.float32)
    rT[:, :A] = rec_w.T
    br = np.zeros((RPAD,), np.float32)
    br[:A] = rec_b
    m = {
        "act": act,
        "b1": c(enc1_b),
        "b2": c(enc2_b),
        "bm": c(mu_b),
        "embTf": c(emb.T),
        "embN": c(emb),
        "d1T": c(dec1_w.T),
        "bd1": c(dec1_b),
        "d2T": c(dec2_w.T),
        "bd2": c(dec2_b),
        "rT": rT,
        "br": br,
    }
    hi, lo = _split_bf16(c(act.T))
    m["actTh"], m["actTl"] = c(hi), c(lo)
    for nm, w in [("w1T", enc1_w.T), ("w2T", enc2_w.T), ("muT", mu_w.T),
                  ("embT", emb.T)]:
        hi, lo = _split_bf16(c(w.astype(np.float32)))
        m[nm + "h"], m[nm + "l"] = c(hi), c(lo)
    return m


def kernel(action, enc1_w, enc1_b, enc2_w, enc2_b, mu_w, mu_b, emb,
           dec1_w, dec1_b, dec2_w, dec2_b, rec_w, rec_b,
           _trace=False, _result_hook=None):
    from concourse.bass_utils import run_bass_kernel_spmd

    if BLOC not in _NC_CACHE:
        _NC_CACHE[BLOC] = _build_nc(BLOC)
    nc = _NC_CACHE[BLOC]

    args = (action, enc1_w, enc1_b, enc2_w, enc2_b, mu_w, mu_b, emb,
            dec1_w, dec1_b, dec2_w, dec2_b, rec_w, rec_b)
    in_maps = [_make_in_map(core, *args) for core in range(NCORES)]
    res = run_bass_kernel_spmd(nc, in_maps, core_ids=list(range(NCORES)),
                               trace=_trace)
    if _result_hook is not None:
        _result_hook(res)

    idx = np.concatenate([res.results[c]["idx_o"] for c in range(NCORES)])
    q_st = np.concatenate([res.results[c]["q_o"] for c in range(NCORES)])
    par = np.stack([res.results[c]["partials"] for c in range(NCORES)])
    # partials: [:, :, 0]=sum(enc^2), [:, :, 1]=sum(maxval), [:, :, 2]=sum((rec-act)^2)
    esum = float(par[:, :, 0].astype(np.float64).sum())
    msum = float(par[:, :, 1].astype(np.float64).sum())
    rsum = float(par[:, :, 2].astype(np.float64).sum())
    vq_sum = esum - 2.0 * msum          # sum over rows of min squared distance
    recons_loss = rsum / (B_TOTAL * A)
    vq_loss = (1.0 + BETA) * vq_sum / (B_TOTAL * D)
    total_loss = np.float32(recons_loss + vq_loss)
    return idx.astype(np.int32), q_st.astype(np.float32), total_loss
